# revision 12
# baseline (speedup 1.0000x reference)
"""Trainium2 Bass kernel for nn_Evolution_26697516712465 (deep-snake GNN).

Self-contained: takes FULL inputs, shards batch across 8 NeuronCores internally
(one image per core; each core runs the snake for the polys of its own image),
returns FULL output [128, 128, 2] fp32.

fp8 DoubleRow design: all large matmuls run in fp8e4 with MatmulPerfMode.
DoubleRow (two K-tiles per instruction at 0.5 cycles/output-column).  All
tensors carry power-of-two scale factors folded into weights host-side; the
final output is rescaled exactly in fp32.

Layouts: conv1/conv2 use linear-130 image positions (L = 130*y + x); the
snake state is vertex-major [128ch, layer, PADV vertex, PADQ poly] so all
matmul moving APs collapse to 3 dims (pair, merged cols).
"""
import numpy as np
import ml_dtypes
from contextlib import ExitStack

import concourse.bass as bass
import concourse.bacc as bacc
import concourse.mybir as mybir
import concourse.tile as tile
from concourse.library_config import mlp as mlp_lib
from concourse.bass_utils import run_bass_kernel_spmd

N_CORES = 8
B, C_IN, H, W = 8, 66, 128, 128
NP, V = 128, 128
RO = 4.0
DIL = (1, 1, 1, 2, 2, 4, 4)
NRES = 7
HW = H * W          # 16384
PADW = W + 2        # 130
PIMG = PADW * PADW  # 16900
NL = 16896          # 33 linear-position tiles of 512
NF = 16640          # feat rows (64 blocks * 260)
PADV = 160          # 16 + 128 + 16 circular pad
NCHUNK = 13         # stack DMA chunks (PIMG = 13*1300)

# power-of-two scale factors (stored = true * S)
S_W1 = 32.0         # conv1 psum scale
S_R1 = 64.0         # relu1
S_F = 4096.0        # feat (psum2 = w2 stored scale)
S_V = 512.0         # vert features in contrib
S_C = 32.0          # coords in contrib
S_HEAD = 8192.0     # head conv psum
S_S = 256.0         # snake states
S_RES = 8192.0      # res conv psum
S_FUS = 16384.0     # fusion psum
S_G = 1024.0        # gmax
S_P1 = 16384.0      # pred1 psum
S_H1 = 1024.0       # h1
S_P2 = 65536.0      # pred2 psum
S_H2 = 8192.0       # h2
S_P3 = 1048576.0    # pred3 psum

f32 = mybir.dt.float32
bf16 = mybir.dt.bfloat16
fp8 = mybir.dt.float8e4
i16 = mybir.dt.int16
AF = mybir.ActivationFunctionType
ALU = mybir.AluOpType
DR = mybir.MatmulPerfMode.DoubleRow

BF = ml_dtypes.bfloat16
E4 = ml_dtypes.float8_e4m3


def _bcast(ap_obj, n):
    """Append a step-0 (broadcast) innermost free dim of size n to an AP."""
    return bass.AP(tensor=ap_obj.tensor, offset=ap_obj.offset,
                   ap=[*ap_obj.ap, [0, n]])


def build_nc(P, bn_extra=False, use_b2s=False, c1bias=False, sb0=True):
    """Build the SPMD Bass program. P = max polys per image."""
    nc = bacc.Bacc("TRN2", target_bir_lowering=False, debug=False)
    PADQ = -(-P // 4) * 4  # snake poly slots (multiple of 4)
    NVI = PADQ * 128       # feat-gather idx count per y-corner
    NII = PADQ * PADV      # ipad gather idx count
    SV = PADV * PADQ       # per-layer state size
    CV = 128 * PADQ        # valid state columns per layer
    CB = 16 * PADQ         # columns per vertex block (<= 512)

    # ---------------- inputs ----------------
    d_st = nc.declare_dram_parameter("st", [99, 2, PIMG], fp8, isOutput=False)
    d_w1p = nc.declare_dram_parameter("w1p", [99, 2, 3, 2, 128], fp8, isOutput=False)
    d_w2t = nc.declare_dram_parameter("w2t", [128, 2, 64], fp8, isOutput=False)
    d_pb0 = nc.declare_dram_parameter("pb0", [128, 2], f32, isOutput=False)
    d_idx0 = nc.declare_dram_parameter("idx0", [128, NVI // 16], i16, isOutput=False)
    d_idx1 = nc.declare_dram_parameter("idx1", [128, NVI // 16], i16, isOutput=False)
    d_wc = nc.declare_dram_parameter("wc", [128, 2, 2, PADQ], f32, isOutput=False)
    d_coords = nc.declare_dram_parameter("coords", [128, PADQ, 2], bf16, isOutput=False)
    d_iidx = nc.declare_dram_parameter("iidx", [128, NII // 16], i16, isOutput=False)
    d_base = nc.declare_dram_parameter("base", [128, PADQ, 2], f32, isOutput=False)
    d_headw = nc.declare_dram_parameter("headw", [33, 9, 2, 128], fp8, isOutput=False)
    d_headb = nc.declare_dram_parameter("headb", [128, 1], f32, isOutput=False)
    d_resw = nc.declare_dram_parameter("resw", [128, 7, 5, 2, 128], fp8, isOutput=False)
    d_resb = nc.declare_dram_parameter("resb", [128, 7], f32, isOutput=False)
    d_fusw = nc.declare_dram_parameter("fusw", [128, 4, 2, 2, 128], fp8, isOutput=False)
    d_fusb = nc.declare_dram_parameter("fusb", [128, 2], f32, isOutput=False)
    d_pw1 = nc.declare_dram_parameter("pw1", [128, 5, 2, 2, 128], fp8, isOutput=False)
    d_pb1 = nc.declare_dram_parameter("pb1", [128, 2], f32, isOutput=False)
    d_pw2 = nc.declare_dram_parameter("pw2", [128, 2, 64], fp8, isOutput=False)
    d_pb2 = nc.declare_dram_parameter("pb2", [64, 1], f32, isOutput=False)
    d_pw3 = nc.declare_dram_parameter("pw3", [64, 2], fp8, isOutput=False)
    if use_b2s:
        d_b2s = nc.declare_dram_parameter("b2s", [128, PADQ, 64], f32, isOutput=False)
    if bn_extra:
        d_bng = nc.declare_dram_parameter("bng", [128, 8], f32, isOutput=False)
        d_bnb = nc.declare_dram_parameter("bnb", [128, 8], f32, isOutput=False)
    d_out = nc.declare_dram_parameter("out", [128, PADQ, 2], f32, isOutput=True)

    feat_dram = nc.dram_tensor("feat_dram", [NF, 64], f32)

    with tile.TileContext(nc, num_cores=N_CORES) as tc, ExitStack() as top:
        wpool = top.enter_context(tc.tile_pool(name="weights", bufs=1))
        # small early-needed tiles on the Act DMA queue
        w2t_t = wpool.tile([128, 2, 64], fp8)
        nc.scalar.dma_start(out=w2t_t, in_=d_w2t[:, :, :])
        pb0_t = wpool.tile([128, 2], f32)
        nc.scalar.dma_start(out=pb0_t, in_=d_pb0[:, :])
        idx0_t = wpool.tile([128, NVI // 16], i16)
        nc.gpsimd.dma_start(out=idx0_t, in_=d_idx0[:, :])
        idx1_t = wpool.tile([128, NVI // 16], i16)
        nc.gpsimd.dma_start(out=idx1_t, in_=d_idx1[:, :])
        wc_t = wpool.tile([128, 2, 2, PADQ], f32)
        nc.gpsimd.dma_start(out=wc_t, in_=d_wc[:, :, :, :])
        coords_t = wpool.tile([128, PADQ, 2], bf16)
        nc.gpsimd.dma_start(out=coords_t, in_=d_coords[:, :, :])
        iidx_t = wpool.tile([128, NII // 16], i16)
        nc.gpsimd.dma_start(out=iidx_t, in_=d_iidx[:, :])
        base_t = wpool.tile([128, PADQ, 2], f32)
        nc.gpsimd.dma_start(out=base_t, in_=d_base[:, :, :])
        if use_b2s:
            b2s_t = wpool.tile([128, PADQ, 64], f32)
            nc.gpsimd.dma_start(out=b2s_t, in_=d_b2s[:, :, :])
        # snake weights (loaded late in program order; declared here)
        headw_t = wpool.tile([33, 9, 2, 128], fp8)
        headb_t = wpool.tile([128, 1], f32)
        resw_t = wpool.tile([128, 7, 5, 2, 128], fp8)
        resb_t = wpool.tile([128, 7], f32)
        fusw_t = wpool.tile([128, 4, 2, 2, 128], fp8)
        fusb_t = wpool.tile([128, 2], f32)
        pw1_t = wpool.tile([128, 5, 2, 2, 128], fp8)
        pb1_t = wpool.tile([128, 2], f32)
        pw2_t = wpool.tile([128, 2, 64], fp8)
        pb2_t = wpool.tile([64, 1], f32)
        pw3_t = wpool.tile([64, 2], fp8)
        if bn_extra:
            bng_t = wpool.tile([128, 8], f32)
            nc.gpsimd.dma_start(out=bng_t, in_=d_bng[:, :])
            bnb_t = wpool.tile([128, 8], f32)
            nc.gpsimd.dma_start(out=bnb_t, in_=d_bnb[:, :])

        contrib = wpool.tile([128, PADQ, 256], fp8)

        nc.gpsimd.load_library(mlp_lib)
        # zero the unused contrib channels early (Pool is idle during conv1)
        nc.gpsimd.memset(contrib[:, :, 64:256], 0.0)
        # warm up the Relu activation table off the critical path
        warm = wpool.tile([128, 1], f32)
        nc.scalar.activation(warm, pb0_t[:, 0:1], AF.Relu)

        # ------------ conv1 (3x3 66->256 fp8 DR) + conv2 (1x1 256->64) ------------
        # linear-position tiles: out position L = 130*y + x (x<128 valid)
        with tc.tile_pool(name="stacks", bufs=1) as stpool, \
             tc.tile_pool(name="psumA", bufs=4, space="PSUM") as ppA, \
             tc.tile_pool(name="psumB", bufs=2, space="PSUM") as ppB, \
             tc.tile_pool(name="stage", bufs=3) as spool:
            st_t = stpool.tile([99, 2, PIMG], fp8)
            CK = PIMG // NCHUNK
            for c in range(NCHUNK):
                nc.sync.dma_start(out=st_t[:, :, c * CK:(c + 1) * CK],
                                  in_=d_st[:, :, c * CK:(c + 1) * CK])
            w1p_t = stpool.tile([99, 2, 3, 2, 128], fp8)
            nc.scalar.dma_start(out=w1p_t, in_=d_w1p[:, :, :, :, :])
            r1 = stpool.tile([128, 2, NL], fp8)
            nc.sync.dma_start(out=headw_t, in_=d_headw[:, :, :, :])
            nc.sync.dma_start(out=headb_t, in_=d_headb[:, :])
            nc.sync.dma_start(out=resw_t[:, 0:4], in_=d_resw[:, 0:4])
            nc.sync.dma_start(out=resw_t[:, 4:7], in_=d_resw[:, 4:7])
            nc.sync.dma_start(out=resb_t, in_=d_resb[:, :])
            nc.sync.dma_start(out=fusw_t, in_=d_fusw[:, :, :, :, :])
            nc.sync.dma_start(out=fusb_t, in_=d_fusb[:, :])
            nc.sync.dma_start(out=pw1_t, in_=d_pw1[:, :, :, :, :])
            nc.sync.dma_start(out=pb1_t, in_=d_pb1[:, :])
            nc.sync.dma_start(out=pw2_t, in_=d_pw2[:, :, :])
            nc.sync.dma_start(out=pb2_t, in_=d_pb2[:, :])
            nc.sync.dma_start(out=pw3_t, in_=d_pw3[:, :])

            def conv2_group(g):
                # 4 two-row px blocks; psum partition p = px pair rows
                # (260*b + 2p, +1) of feat_dram
                ps2 = ppB.tile([128, 512], f32, tag="psB", name="psB")
                for j in range(4):
                    b = 4 * g + j
                    for par in range(2):
                        lhsT = bass.AP(tensor=r1.tensor,
                                       offset=r1.offset + 260 * b + par,
                                       ap=[r1.ap[0], [NL, 2], [2, 128]])
                        nc.tensor.matmul(ps2[:, j * 128 + par * 64:
                                             j * 128 + par * 64 + 64],
                                         lhsT, w2t_t[:, :, :],
                                         start=True, stop=True, perf_mode=DR)
                stg = spool.tile([128, 512], f32, tag="stage", name="stg")
                if g % 2 == 0:
                    nc.scalar.activation(stg, ps2, AF.Copy)
                else:
                    nc.vector.tensor_copy(stg, ps2)
                dst = bass.AP(tensor=feat_dram, offset=g * 4 * 260 * 64,
                              ap=[[128, 128], [260 * 64, 4], [1, 128]])
                nc.sync.dma_start(out=dst, in_=stg)

            for t in range(33):          # linear tiles of 512 positions
                L0 = 512 * t
                for m in range(2):       # out-channel half
                    ps = ppA.tile([128, 512], f32, tag="psA", name="psA")
                    for kw in range(3):
                        rhs = bass.AP(tensor=st_t.tensor,
                                      offset=st_t.offset + L0 + kw,
                                      ap=[st_t.ap[0], [PIMG, 2], [1, 512]])
                        nc.tensor.matmul(ps, w1p_t[:, :, kw, m, :], rhs,
                                         start=(kw == 0), stop=(kw == 2),
                                         perf_mode=DR)
                    dst = r1[:, m, L0:L0 + 512]
                    ri = (t * 2 + m) % 11
                    if c1bias or ri < 5:
                        nc.scalar.activation(dst, ps, AF.Relu,
                                             bias=pb0_t[:, m:m + 1], scale=2.0)
                    elif ri < 9:
                        nc.vector.tensor_scalar(dst, ps, 2.0, 0.0,
                                                op0=ALU.mult, op1=ALU.max)
                    else:
                        nc.gpsimd.tensor_scalar(dst, ps, 2.0, 0.0,
                                                op0=ALU.mult, op1=ALU.max)
                # conv2 group g ready once relu covers 1040*g + 1041 positions
                if t >= 3 and t % 2 == 1:
                    conv2_group((t - 3) // 2)
            for g in range(15, 16):
                conv2_group(g)
            # catch-up: the 4 positions per 260-block not covered by M=[2,128]
            psx = ppB.tile([64, 256], f32, tag="psX", name="psX")
            for z, Lo in enumerate((256, 258)):
                for par in range(2):
                    lhsT = bass.AP(tensor=r1.tensor,
                                   offset=r1.offset + Lo + par,
                                   ap=[r1.ap[0], [NL, 2], [260, 64]])
                    nc.tensor.matmul(psx[:, (2 * z + par) * 64:
                                         (2 * z + par) * 64 + 64],
                                     lhsT, w2t_t[:, :, :],
                                     start=True, stop=True, perf_mode=DR)
            stg2 = spool.tile([64, 256], f32, tag="stage2", name="stg2")
            nc.scalar.activation(stg2, psx, AF.Copy)
            dstx = bass.AP(tensor=feat_dram, offset=256 * 64,
                           ap=[[260 * 64, 64], [128, 2], [1, 128]])
            nc.sync.dma_start(out=dstx, in_=stg2)

        # ------------ bilinear gather (2 row-pair gathers) + combine ------------
        with tc.tile_pool(name="gpool", bufs=1) as gpool:
            gts = []
            src = bass.AP(tensor=feat_dram, offset=0, ap=[[64, NF - 1], [1, 128]])
            for yi, idxt in enumerate((idx0_t, idx1_t)):
                gt = gpool.tile([128, PADQ, 128], f32, tag=f"g{yi}", name=f"g{yi}")
                nc.gpsimd.dma_gather(gt, src, idxt[:, :], NVI, NVI, 128,
                                     elem_step=64, single_packet=False)
                gts.append(gt)
            va = gpool.tile([128, PADQ, 64], bf16, tag="va", name="va")
            vb = gpool.tile([128, PADQ, 64], bf16, tag="vb", name="vb")
            nc.vector.tensor_tensor(va, gts[0][:, :, 0:64],
                                    _bcast(wc_t[:, 0, 0, :], 64), ALU.mult)
            nc.vector.tensor_tensor(vb, gts[0][:, :, 64:128],
                                    _bcast(wc_t[:, 0, 1, :], 64), ALU.mult)
            nc.vector.tensor_tensor(va, va, vb, ALU.add)
            nc.vector.tensor_tensor(vb, gts[1][:, :, 0:64],
                                    _bcast(wc_t[:, 1, 0, :], 64), ALU.mult)
            nc.vector.tensor_tensor(va, va, vb, ALU.add)
            nc.vector.tensor_tensor(vb, gts[1][:, :, 64:128],
                                    _bcast(wc_t[:, 1, 1, :], 64), ALU.mult)
            if use_b2s:
                nc.vector.tensor_tensor(vb, vb, b2s_t, ALU.add)
            nc.vector.tensor_tensor(contrib[:, :, 0:64], va, vb, ALU.add)
            nc.vector.tensor_copy(contrib[:, :, 64:66], coords_t)

            # ---------------- snake (vertex-major: [ch, layer, k, q]) ----------
            with tc.tile_pool(name="snake", bufs=1) as sn, \
                 tc.tile_pool(name="psumS", bufs=4, space="PSUM") as ppS, \
                 tc.tile_pool(name="psumT", bufs=2, space="PSUM") as ppT:
                # transpose-gather contrib -> ipad (memory [128, NII, 2] fp8:
                # partition p plane j = channel 2p+j of position n = k*PADQ+q)
                ipad = sn.tile([128, 2, NII], fp8, tag="ipad", name="ipad")
                NI1 = 96 * PADQ
                for n0, n1 in ((0, NI1), (NI1, NII)):
                    odst = bass.AP(tensor=ipad.tensor,
                                   offset=ipad.offset + 2 * n0,
                                   ap=[ipad.ap[0], [n1 - n0, 2], [1, n1 - n0]])
                    nc.gpsimd.dma_gather(odst, contrib[:, :, :],
                                         iidx_t[:, 16 * n0 // 256:16 * n1 // 256],
                                         n1 - n0, n1 - n0, 256, transpose=True,
                                         single_packet=False,
                                         sbuf_tokens_per_rank=128,
                                         sbuf_free_dim_per_rank=256,
                                         sbuf_byte_offset=0)

                states = sn.tile([128, 8, PADV, PADQ], fp8, tag="states",
                                 name="states")
                ctrs = [sn.tile([128, 8, CB], fp8, tag=f"ctr{k}",
                                name=f"ctr{k}") for k in range(2)]

                def st_slice(li, k0, k1):
                    """states[:, li, k0:k1, :] as [128, (k1-k0)*PADQ]."""
                    return bass.AP(
                        tensor=states.tensor,
                        offset=states.offset + (li * PADV + k0) * PADQ,
                        ap=[states.ap[0], [1, (k1 - k0) * PADQ]])

                # head conv: K = 33 partitions x 2 (interleaved channel pairs)
                for vb in range(8):
                    ps = ppS.tile([128, CB], f32, tag="psS", name="psS")
                    for t in range(9):
                        off = 2 * ((16 + vb * 16 + (t - 4)) * PADQ)
                        rhs = bass.AP(tensor=ipad.tensor,
                                      offset=ipad.offset + off,
                                      ap=[[ipad.ap[0][0], 33], [1, 2],
                                          [2, CB]])
                        nc.tensor.matmul(ps, headw_t[:, t, :, :], rhs,
                                         start=(t == 0), stop=(t == 8),
                                         perf_mode=DR)
                    dst = st_slice(0, 16 + vb * 16, 32 + vb * 16)
                    if bn_extra:
                        cv = ctrs[0][:, vb, :]
                        nc.scalar.activation(cv, ps, AF.Relu,
                                             bias=headb_t[:, 0:1],
                                             scale=S_S / S_HEAD)
                        nc.vector.tensor_scalar(dst, cv,
                                                bng_t[:, 0:1], bnb_t[:, 0:1],
                                                op0=ALU.mult, op1=ALU.add)
                    else:
                        nc.scalar.activation(dst, ps, AF.Relu,
                                             bias=headb_t[:, 0:1],
                                             scale=S_S / S_HEAD)
                    if vb == 7:
                        nc.gpsimd.tensor_copy(st_slice(0, 0, 16),
                                              st_slice(0, 128, 144))
                    if vb == 0:
                        nc.gpsimd.tensor_copy(st_slice(0, 144, 160),
                                              st_slice(0, 16, 32))

                # res convs: 5 DR tap-pairs, fp8 states
                for i in range(NRES):
                    d = DIL[i]
                    ctr = ctrs[i % 2]
                    for jj in range(8):
                        vb = (i + 1 + jj) % 8   # rotate start for pipelining
                        ps = ppS.tile([128, CB], f32, tag="psS", name="psS")
                        for ti, t0 in enumerate((0, 2, 4, 6, 8)):
                            stride = d if t0 != 8 else 0
                            k0 = 16 + vb * 16 + (t0 - 4) * d
                            off = (i * PADV + k0) * PADQ
                            rhs = bass.AP(tensor=states.tensor,
                                          offset=states.offset + off,
                                          ap=[states.ap[0],
                                              [stride * PADQ, 2], [1, CB]])
                            nc.tensor.matmul(ps, resw_t[:, i, ti, :, :], rhs,
                                             start=(ti == 0), stop=(ti == 4),
                                             perf_mode=DR)
                        cv = ctr[:, vb, :]
                        if sb0 and jj >= 6:
                            nc.vector.tensor_scalar(cv, ps, S_S / S_RES,
                                                    0.0, op0=ALU.mult,
                                                    op1=ALU.max)
                        else:
                            nc.scalar.activation(cv, ps, AF.Relu,
                                                 bias=resb_t[:, i:i + 1],
                                                 scale=S_S / S_RES)
                        if bn_extra:
                            nc.vector.tensor_scalar(cv, cv, bng_t[:, i + 1:i + 2],
                                                    bnb_t[:, i + 1:i + 2],
                                                    op0=ALU.mult, op1=ALU.add)
                        radd = nc.gpsimd if jj % 2 == 0 else nc.vector
                        radd.tensor_tensor(
                            st_slice(i + 1, 16 + vb * 16, 32 + vb * 16), cv,
                            st_slice(i, 16 + vb * 16, 32 + vb * 16), ALU.add)
                        if vb == 7:
                            nc.gpsimd.tensor_copy(st_slice(i + 1, 0, 16),
                                                  st_slice(i + 1, 128, 144))
                        if vb == 0:
                            nc.gpsimd.tensor_copy(st_slice(i + 1, 144, 160),
                                                  st_slice(i + 1, 16, 32))

                # fusion 1x1 (1024->256) + per-poly max over V
                gmv = sn.tile([128, 2, 8, PADQ], f32, tag="gmv", name="gmv")
                gb = sn.tile([128, 2, PADQ], fp8, tag="gb", name="gb")
                gbx = sn.tile([128, 2, CB], fp8, tag="gbx", name="gbx")
                for m in range(2):
                    for vb in range(8):
                        ps = ppS.tile([128, CB], f32, tag="psS", name="psS")
                        for u in range(4):
                            off = (2 * u * PADV + 16 + vb * 16) * PADQ
                            rhs = bass.AP(tensor=states.tensor,
                                          offset=states.offset + off,
                                          ap=[states.ap[0], [SV, 2], [1, CB]])
                            nc.tensor.matmul(ps, fusw_t[:, u, :, m, :], rhs,
                                             start=(u == 0), stop=(u == 3),
                                             perf_mode=DR)
                        nc.vector.tensor_reduce(
                            gmv[:, m, vb, :],
                            bass.AP(tensor=ps.tensor, offset=ps.offset,
                                    ap=[ps.ap[0], [1, PADQ], [PADQ, 16]]),
                            axis=mybir.AxisListType.X, op=ALU.max)
                    # tree max over the 8 vertex blocks
                    nc.vector.tensor_tensor(gmv[:, m, 0:4, :], gmv[:, m, 0:4, :],
                                            gmv[:, m, 4:8, :], ALU.max)
                    nc.vector.tensor_tensor(gmv[:, m, 0:2, :], gmv[:, m, 0:2, :],
                                            gmv[:, m, 2:4, :], ALU.max)
                    nc.vector.tensor_tensor(gmv[:, m, 0:1, :], gmv[:, m, 0:1, :],
                                            gmv[:, m, 1:2, :], ALU.max)
                    nc.vector.tensor_scalar(gb[:, m, :], gmv[:, m, 0, :],
                                            S_G / S_FUS, fusb_t[:, m:m + 1],
                                            op0=ALU.mult, op1=ALU.add)
                    nc.vector.tensor_copy(
                        gbx[:, m, :],
                        bass.AP(tensor=gb.tensor, offset=gb.offset + m * PADQ,
                                ap=[gb.ap[0], [0, 16], [1, PADQ]]))

                # pred1: 1280 -> 256, relu -> h1 fp8 (vertex-major)
                h1 = sn.tile([128, 2, CV], fp8, tag="h1", name="h1")
                for m in range(2):
                    for vb in range(8):
                        ps = ppS.tile([128, CB], f32, tag="psS", name="psS")
                        rhs = bass.AP(tensor=gbx.tensor, offset=gbx.offset,
                                      ap=[gbx.ap[0], [CB, 2], [1, CB]])
                        nc.tensor.matmul(ps, pw1_t[:, 0, :, m, :], rhs,
                                         start=True, stop=False, perf_mode=DR)
                        for u in range(4):
                            off = (2 * u * PADV + 16 + vb * 16) * PADQ
                            rhs = bass.AP(tensor=states.tensor,
                                          offset=states.offset + off,
                                          ap=[states.ap[0], [SV, 2], [1, CB]])
                            nc.tensor.matmul(ps, pw1_t[:, u + 1, :, m, :], rhs,
                                             start=False, stop=(u == 3),
                                             perf_mode=DR)
                        if sb0 and vb % 8 >= 5:
                            nc.vector.tensor_scalar(
                                h1[:, m, vb * CB:(vb + 1) * CB], ps,
                                S_H1 / S_P1, 0.0,
                                op0=ALU.mult, op1=ALU.max)
                        else:
                            nc.scalar.activation(
                                h1[:, m, vb * CB:(vb + 1) * CB], ps, AF.Relu,
                                bias=pb1_t[:, m:m + 1], scale=S_H1 / S_P1)

                # pred2: 256 -> 64, relu -> h2 fp8
                h2 = sn.tile([64, CV], fp8, tag="h2", name="h2")
                for vb in range(8):
                    ps = ppT.tile([64, CB], f32, tag="psS2", name="psS2")
                    rhs = bass.AP(tensor=h1.tensor,
                                  offset=h1.offset + vb * CB,
                                  ap=[h1.ap[0], [CV, 2], [1, CB]])
                    nc.tensor.matmul(ps, pw2_t[:, :, :], rhs,
                                     start=True, stop=True, perf_mode=DR)
                    nc.scalar.activation(h2[:, vb * CB:(vb + 1) * CB], ps,
                                         AF.Relu, bias=pb2_t[:, 0:1],
                                         scale=S_H2 / S_P2)

                # pred3: 64 -> 2 per poly -> [128 v, PADQ, 2]
                ps3 = ppT.tile([128, PADQ * 2], f32, tag="psT3", name="psT3",
                               bufs=1)
                for q in range(PADQ):
                    lhsT = bass.AP(tensor=h2.tensor, offset=h2.offset + q,
                                   ap=[h2.ap[0], [PADQ, 128]])
                    nc.tensor.matmul(ps3[:, q * 2:(q + 1) * 2],
                                     lhsT, pw3_t[:, :], start=True, stop=True)
                o_t = sn.tile([128, PADQ, 2], f32, tag="o_t", name="o_t")
                nc.vector.scalar_tensor_tensor(
                    o_t, ps3.rearrange("p (a b) -> p a b", b=2), 1.0 / S_P3,
                    base_t, op0=ALU.mult, op1=ALU.add)
                nc.sync.dma_start(out=d_out[:, :, :], in_=o_t)

    nc.compile()
    return nc


_NC_CACHE = {}


def _get_nc(P, bn_extra=False, use_b2s=False, c1bias=False, sb0=True):
    key = (P, bn_extra, use_b2s, c1bias, sb0)
    if key not in _NC_CACHE:
        _NC_CACHE[key] = build_nc(P, bn_extra, use_b2s, c1bias, sb0)
    return _NC_CACHE[key]


def _flags(inputs):
    g_ok = (np.all(np.asarray(inputs["head_g"]) > 0)
            and np.all(np.asarray(inputs["res_g"]) > 0))
    b_zero = (np.all(np.asarray(inputs["head_bt"]) == 0)
              and np.all(np.asarray(inputs["res_bt"]) == 0))
    bn_extra = not (g_ok and b_zero)
    use_b2s = bool(np.any(np.asarray(inputs["proj_b2"]) != 0))
    c1bias = bool(np.any(np.asarray(inputs["proj_b1"]) != 0))
    sb0 = (np.all(np.asarray(inputs["head_b"]) == 0)
           and np.all(np.asarray(inputs["res_b"]) == 0)
           and np.all(np.asarray(inputs["pb1"]) == 0))
    return bn_extra, use_b2s, c1bias, sb0


def pack16(idx_flat, cols):
    tab = np.zeros((16, cols), np.int16)
    n = len(idx_flat)
    tab[np.arange(n) % 16, np.arange(n) // 16] = idx_flat.astype(np.int16)
    return np.tile(tab, (8, 1))


def _host_prep(inputs, P, counts, order, offs):
    """Build per-core in_maps."""
    bn_extra, use_b2s, c1bias, sb0 = _flags(inputs)
    cnn = np.asarray(inputs["cnn_feature"], np.float32)
    ipoly = np.asarray(inputs["i_it_poly"], np.float32)
    cpoly = np.asarray(inputs["c_it_poly"], np.float32)
    w1 = np.asarray(inputs["proj_w1"], np.float32)
    b1 = np.asarray(inputs["proj_b1"], np.float32)
    b2 = np.asarray(inputs["proj_b2"], np.float32)
    w2 = np.asarray(inputs["proj_w2"], np.float32)[:, :, 0, 0]  # [64, 256]
    PADQ = -(-P // 4) * 4
    NVI = PADQ * 128
    NII = PADQ * PADV

    # ---- grid-sample host math (fp32, matches reference) ----
    ix = ipoly[..., 0] - np.float32(0.5)
    iy = ipoly[..., 1] - np.float32(0.5)
    x0 = np.floor(ix); y0 = np.floor(iy)
    wx = (ix - x0).astype(np.float32); wy = (iy - y0).astype(np.float32)
    x0i = x0.astype(np.int64); y0i = y0.astype(np.int64)
    bx = np.clip(x0i, 0, W - 2)                      # gather row base (x)
    # feat rows are linear positions L = 130*y + x
    rows_y = []
    wslot = np.zeros((2, 2) + ipoly.shape[:2], np.float32)  # [yi, slot, NP, V]
    s_v = np.zeros(ipoly.shape[:2], np.float32)
    for yi in range(2):
        yc = y0i + yi
        yv = (yc >= 0) & (yc < H)
        ycc = np.clip(yc, 0, H - 1)
        rows_y.append(ycc * PADW + bx)
        wgt_y = np.where(yv, wy if yi else 1 - wy, 0.0)
        for s in range(2):
            xs = bx + s
            is_x0 = (xs == x0i) & (x0i >= 0)
            is_x1 = (xs == x0i + 1) & (x0i + 1 <= W - 1)
            wgt_x = np.where(is_x0, 1 - wx, np.where(is_x1, wx, 0.0))
            wslot[yi, s] = wgt_y * wgt_x
        s_v += wslot[yi, 0] + wslot[yi, 1]

    # ---- shared packed weights (fp8 with scale folding) ----
    def row_kh_ch(r):
        if r < 66:
            return 0, r
        if r < 132:
            return 1, r - 66
        return 2, r - 132
    w1p = np.zeros((99, 2, 3, 2, 128), np.float32)
    for r in range(198):
        p, j = r % 99, r // 99
        kh, ci = row_kh_ch(r)
        for kw in range(3):
            for m in range(2):
                w1p[p, j, kw, m, :] = w1[m * 128:(m + 1) * 128, ci, kh, kw] * S_W1
    w2t = (w2.T * (S_F / S_R1)).reshape(2, 128, 64).transpose(1, 0, 2)
    pb0 = (b1 * S_R1).reshape(2, 128).T.copy()

    head_g = np.asarray(inputs["head_g"], np.float32)
    res_g = np.asarray(inputs["res_g"], np.float32)
    gfold_h = head_g if not bn_extra else np.ones_like(head_g)
    gfold_r = res_g if not bn_extra else np.ones_like(res_g)

    # head: [33 part, 9 taps, 2 pair, 128 out]; channel of (p, j) = 2p+j
    hw_true = np.asarray(inputs["head_w"], np.float32)  # [128, 66, 9]
    sch = np.where(np.arange(66) < 64, S_V, S_C)
    headw = np.zeros((33, 9, 2, 128), np.float32)
    for p in range(33):
        for j in range(2):
            c = 2 * p + j
            headw[p, :, j, :] = (hw_true[:, c, :] * (S_HEAD / sch[c])
                                 * gfold_h[:, None]).T
    headb = (np.asarray(inputs["head_b"], np.float32) * S_S).reshape(128, 1)

    # res: tap pairs (0,1),(2,3),(4,5),(6,7),(8,dup-zero)
    rw_true = np.asarray(inputs["res_w"], np.float32)  # [7, 128, 128, 9]
    resw = np.zeros((128, 7, 5, 2, 128), np.float32)
    for i in range(7):
        wi = rw_true[i] * gfold_r[i][:, None, None]  # [o, c, tap]
        for ti, t0 in enumerate((0, 2, 4, 6, 8)):
            resw[:, i, ti, 0, :] = wi[:, :, t0].T * (S_RES / S_S)
            if t0 != 8:
                resw[:, i, ti, 1, :] = wi[:, :, t0 + 1].T * (S_RES / S_S)
    resb = (np.asarray(inputs["res_b"], np.float32) * S_S).T.copy()

    fw = np.asarray(inputs["fus_w"], np.float32).reshape(256, 8, 128)
    fusw = np.zeros((128, 4, 2, 2, 128), np.float32)
    for u in range(4):
        for j in range(2):
            for m in range(2):
                fusw[:, u, j, m, :] = (fw[m * 128:(m + 1) * 128, 2 * u + j, :]
                                       * (S_FUS / S_S)).T
    fusb = (np.asarray(inputs["fus_b"], np.float32) * S_G).reshape(2, 128).T.copy()

    p1 = np.asarray(inputs["pw1"], np.float32).reshape(256, 10, 128)
    pw1 = np.zeros((128, 5, 2, 2, 128), np.float32)
    for m in range(2):
        for j in range(2):
            pw1[:, 0, j, m, :] = (p1[m * 128:(m + 1) * 128, j, :]
                                  * (S_P1 / S_G)).T
        for u in range(4):
            for j in range(2):
                pw1[:, u + 1, j, m, :] = (p1[m * 128:(m + 1) * 128, 2 + 2 * u + j, :]
                                          * (S_P1 / S_S)).T
    pb1 = (np.asarray(inputs["pb1"], np.float32) * S_H1).reshape(2, 128).T.copy()
    p2 = np.asarray(inputs["pw2"], np.float32).reshape(64, 2, 128)
    pw2 = (p2 * (S_P2 / S_H1)).transpose(2, 1, 0).copy()
    pb2 = (np.asarray(inputs["pb2"], np.float32) * S_H2).reshape(64, 1)
    pw3 = (np.asarray(inputs["pw3"], np.float32).T * (S_P3 / S_H2)).copy()
    pb3 = np.asarray(inputs["pb3"], np.float32)

    shared = {
        "w1p": w1p.astype(E4), "w2t": w2t.astype(E4), "pb0": pb0,
        "headw": headw.astype(E4), "headb": headb,
        "resw": resw.astype(E4), "resb": resb,
        "fusw": fusw.astype(E4), "fusb": fusb,
        "pw1": pw1.astype(E4), "pb1": pb1,
        "pw2": pw2.astype(E4), "pb2": pb2, "pw3": pw3.astype(E4),
    }
    if bn_extra:
        bng = np.zeros((128, 8), np.float32)
        bnb = np.zeros((128, 8), np.float32)
        bng[:, 0] = head_g; bnb[:, 0] = np.asarray(inputs["head_bt"]) * S_S
        bng[:, 1:] = res_g.T; bnb[:, 1:] = np.asarray(inputs["res_bt"]).T * S_S
        shared["bng"] = bng; shared["bnb"] = bnb

    # iidx: ipad pos n = k*PADQ + q <- contrib token q, vertex (k+112)%128
    kk = np.arange(PADV)
    vv = (kk + 112) % 128
    iidx_full = (np.arange(PADQ)[None, :] * 128 + vv[:, None]).reshape(-1)

    ind = np.asarray(inputs["ind"]).astype(np.int64)
    in_maps = []
    for c in range(N_CORES):
        img = cnn[c]
        img_pad = np.zeros((C_IN, PADW, PADW), np.float32)
        img_pad[:, 1:129, 1:129] = img
        flat = img_pad.reshape(C_IN, PIMG)
        st = np.zeros((99, 2, PIMG), np.float32)
        for r in range(198):
            p, j = r % 99, r // 99
            kh, ci = row_kh_ch(r)
            if kh == 0:
                st[p, j] = flat[ci]
            else:
                st[p, j, :PIMG - 130 * kh] = flat[ci, 130 * kh:]

        own = order[offs[c]:offs[c + 1]]
        nown = len(own)
        idxs = np.zeros((2, NVI), np.int64)
        wc = np.zeros((128, 2, 2, PADQ), np.float32)
        for yi in range(2):
            idxs[yi, :nown * 128] = rows_y[yi][own].reshape(-1)
            for s in range(2):
                wc[:, yi, s, :nown] = (wslot[yi, s][own].T * (S_V / S_F))
        coords = np.zeros((128, PADQ, 2), np.float32)
        coords[:, :nown, :] = (cpoly[own] * (RO * S_C)).transpose(1, 0, 2)
        base = np.zeros((128, PADQ, 2), np.float32)
        if nown:
            base[:, :nown, :] = (ipoly[own] * RO + pb3[None, None, :]) \
                .transpose(1, 0, 2).astype(np.float32)

        m = {
            "st": st.astype(E4),
            "idx0": pack16(idxs[0], NVI // 16),
            "idx1": pack16(idxs[1], NVI // 16),
            "wc": wc, "coords": coords.astype(BF),
            "iidx": pack16(iidx_full, NII // 16),
            "base": base,
        }
        if use_b2s:
            b2s = np.zeros((128, PADQ, 64), np.float32)
            b2s[:, :nown, :] = s_v[own].T[:, :, None] * b2[None, None, :] * S_V
            m["b2s"] = b2s
        m.update(shared)
        in_maps.append(m)
    return in_maps


def kernel(**inputs):
    ind = np.asarray(inputs["ind"]).astype(np.int64)
    counts = np.bincount(ind, minlength=N_CORES)
    P = int(counts.max())
    assert P <= 32, f"per-image poly count {P} exceeds supported range"
    order = np.argsort(ind, kind="stable")
    offs = np.concatenate([[0], np.cumsum(counts)])

    bn_extra, use_b2s, c1bias, sb0 = _flags(inputs)
    nc = _get_nc(P, bn_extra, use_b2s, c1bias, sb0)
    in_maps = _host_prep(inputs, P, counts, order, offs)
    res = None
    last_err = None
    for _attempt in range(3):
        try:
            res = run_bass_kernel_spmd(nc, in_maps, list(range(N_CORES)))
            break
        except Exception as e:  # rare transient device error; retry
            last_err = e
    if res is None:
        raise last_err

    out = np.zeros((NP, V, 2), np.float32)
    for c in range(N_CORES):
        oc = res.results[c]["out"]  # [128v, PADQ, 2]
        own = order[offs[c]:offs[c + 1]]
        for q, opoly in enumerate(own):
            out[opoly] = oc[:, q, :]
    return out


# revision 13
# speedup vs baseline: 1.0562x; 1.0562x over previous
"""Trainium2 Bass kernel for nn_Evolution_26697516712465 (deep-snake GNN).

Self-contained: takes FULL inputs, shards batch across 8 NeuronCores internally
(one image per core; each core runs the snake for the polys of its own image),
returns FULL output [128, 128, 2] fp32.

fp8 DoubleRow design: all large matmuls run in fp8e4 with MatmulPerfMode.
DoubleRow (two K-tiles per instruction at 0.5 cycles/output-column).  All
tensors carry power-of-two scale factors folded into weights host-side; the
final output is rescaled exactly in fp32.

Layouts: conv1/conv2 use linear-130 image positions (L = 130*y + x); the
snake state is vertex-major [128ch, layer, PADV vertex, PADQ poly] so all
matmul moving APs collapse to 3 dims (pair, merged cols).
"""
import numpy as np
import ml_dtypes
from contextlib import ExitStack

import concourse.bass as bass
import concourse.bacc as bacc
import concourse.mybir as mybir
import concourse.tile as tile
from concourse.library_config import mlp as mlp_lib
from concourse.bass_utils import run_bass_kernel_spmd

N_CORES = 8
B, C_IN, H, W = 8, 66, 128, 128
NP, V = 128, 128
RO = 4.0
DIL = (1, 1, 1, 2, 2, 4, 4)
NRES = 7
HW = H * W          # 16384
PADW = W + 2        # 130
PIMG = PADW * PADW  # 16900
NL = 16896          # 33 linear-position tiles of 512
NF = 16640          # feat rows (64 blocks * 260)
PADV = 160          # 16 + 128 + 16 circular pad
NCHUNK = 13         # stack DMA chunks (PIMG = 13*1300)

# power-of-two scale factors (stored = true * S)
S_W1 = 32.0         # conv1 psum scale
S_R1 = 64.0         # relu1
S_F = 4096.0        # feat (psum2 = w2 stored scale)
S_V = 512.0         # vert features in contrib
S_C = 32.0          # coords in contrib
S_HEAD = 8192.0     # head conv psum
S_S = 256.0         # snake states
S_RES = 8192.0      # res conv psum
S_FUS = 16384.0     # fusion psum
S_G = 1024.0        # gmax
S_P1 = 16384.0      # pred1 psum
S_H1 = 1024.0       # h1
S_P2 = 65536.0      # pred2 psum
S_H2 = 8192.0       # h2
S_P3 = 1048576.0    # pred3 psum

f32 = mybir.dt.float32
bf16 = mybir.dt.bfloat16
fp8 = mybir.dt.float8e4
i16 = mybir.dt.int16
AF = mybir.ActivationFunctionType
ALU = mybir.AluOpType
DR = mybir.MatmulPerfMode.DoubleRow

BF = ml_dtypes.bfloat16
E4 = ml_dtypes.float8_e4m3


def _bcast(ap_obj, n):
    """Append a step-0 (broadcast) innermost free dim of size n to an AP."""
    return bass.AP(tensor=ap_obj.tensor, offset=ap_obj.offset,
                   ap=[*ap_obj.ap, [0, n]])


def build_nc(P, bn_extra=False, use_b2s=False, c1bias=False, sb0=True):
    """Build the SPMD Bass program. P = max polys per image."""
    nc = bacc.Bacc("TRN2", target_bir_lowering=False, debug=False)
    PADQ = -(-P // 4) * 4  # snake poly slots (multiple of 4)
    NVI = PADQ * 128       # feat-gather idx count per y-corner
    NII = PADQ * PADV      # ipad gather idx count
    SV = PADV * PADQ       # per-layer state size
    CV = 128 * PADQ        # valid state columns per layer
    CB = 16 * PADQ         # columns per vertex block (<= 512)

    # ---------------- inputs ----------------
    d_st = nc.declare_dram_parameter("st", [99, 2, PIMG], fp8, isOutput=False)
    d_w1p = nc.declare_dram_parameter("w1p", [99, 2, 3, 2, 128], fp8, isOutput=False)
    d_w2t = nc.declare_dram_parameter("w2t", [128, 2, 64], fp8, isOutput=False)
    d_pb0 = nc.declare_dram_parameter("pb0", [128, 2], f32, isOutput=False)
    d_idx0 = nc.declare_dram_parameter("idx0", [128, NVI // 16], i16, isOutput=False)
    d_idx1 = nc.declare_dram_parameter("idx1", [128, NVI // 16], i16, isOutput=False)
    d_wc = nc.declare_dram_parameter("wc", [128, 2, 2, PADQ], f32, isOutput=False)
    d_coords = nc.declare_dram_parameter("coords", [128, PADQ, 2], bf16, isOutput=False)
    d_iidx = nc.declare_dram_parameter("iidx", [128, NII // 16], i16, isOutput=False)
    d_base = nc.declare_dram_parameter("base", [128, PADQ, 2], f32, isOutput=False)
    d_headw = nc.declare_dram_parameter("headw", [33, 9, 2, 128], fp8, isOutput=False)
    d_headb = nc.declare_dram_parameter("headb", [128, 1], f32, isOutput=False)
    d_resw = nc.declare_dram_parameter("resw", [128, 7, 5, 2, 128], fp8, isOutput=False)
    d_resb = nc.declare_dram_parameter("resb", [128, 7], f32, isOutput=False)
    d_fusw = nc.declare_dram_parameter("fusw", [128, 4, 2, 2, 128], fp8, isOutput=False)
    d_fusb = nc.declare_dram_parameter("fusb", [128, 2], f32, isOutput=False)
    d_pw1 = nc.declare_dram_parameter("pw1", [128, 5, 2, 2, 128], fp8, isOutput=False)
    d_pb1 = nc.declare_dram_parameter("pb1", [128, 2], f32, isOutput=False)
    d_pw2 = nc.declare_dram_parameter("pw2", [128, 2, 64], fp8, isOutput=False)
    d_pb2 = nc.declare_dram_parameter("pb2", [64, 1], f32, isOutput=False)
    d_pw3 = nc.declare_dram_parameter("pw3", [64, 2], fp8, isOutput=False)
    if use_b2s:
        d_b2s = nc.declare_dram_parameter("b2s", [128, PADQ, 64], f32, isOutput=False)
    if bn_extra:
        d_bng = nc.declare_dram_parameter("bng", [128, 8], f32, isOutput=False)
        d_bnb = nc.declare_dram_parameter("bnb", [128, 8], f32, isOutput=False)
    d_out = nc.declare_dram_parameter("out", [128, PADQ, 2], f32, isOutput=True)

    feat_dram = nc.dram_tensor("feat_dram", [NF, 64], f32)

    with tile.TileContext(nc, num_cores=N_CORES) as tc, ExitStack() as top:
        wpool = top.enter_context(tc.tile_pool(name="weights", bufs=1))
        # small early-needed tiles on the Act DMA queue
        w2t_t = wpool.tile([128, 2, 64], fp8)
        nc.scalar.dma_start(out=w2t_t, in_=d_w2t[:, :, :])
        pb0_t = wpool.tile([128, 2], f32)
        nc.scalar.dma_start(out=pb0_t, in_=d_pb0[:, :])
        idx0_t = wpool.tile([128, NVI // 16], i16)
        nc.gpsimd.dma_start(out=idx0_t, in_=d_idx0[:, :])
        idx1_t = wpool.tile([128, NVI // 16], i16)
        nc.gpsimd.dma_start(out=idx1_t, in_=d_idx1[:, :])
        wc_t = wpool.tile([128, 2, 2, PADQ], f32)
        nc.gpsimd.dma_start(out=wc_t, in_=d_wc[:, :, :, :])
        coords_t = wpool.tile([128, PADQ, 2], bf16)
        nc.gpsimd.dma_start(out=coords_t, in_=d_coords[:, :, :])
        iidx_t = wpool.tile([128, NII // 16], i16)
        nc.gpsimd.dma_start(out=iidx_t, in_=d_iidx[:, :])
        base_t = wpool.tile([128, PADQ, 2], f32)
        nc.gpsimd.dma_start(out=base_t, in_=d_base[:, :, :])
        if use_b2s:
            b2s_t = wpool.tile([128, PADQ, 64], f32)
            nc.gpsimd.dma_start(out=b2s_t, in_=d_b2s[:, :, :])
        # snake weights (loaded late in program order; declared here)
        headw_t = wpool.tile([33, 9, 2, 128], fp8)
        headb_t = wpool.tile([128, 1], f32)
        resw_t = wpool.tile([128, 7, 5, 2, 128], fp8)
        resb_t = wpool.tile([128, 7], f32)
        fusw_t = wpool.tile([128, 4, 2, 2, 128], fp8)
        fusb_t = wpool.tile([128, 2], f32)
        pw1_t = wpool.tile([128, 5, 2, 2, 128], fp8)
        pb1_t = wpool.tile([128, 2], f32)
        pw2_t = wpool.tile([128, 2, 64], fp8)
        pb2_t = wpool.tile([64, 1], f32)
        pw3_t = wpool.tile([64, 2], fp8)
        if bn_extra:
            bng_t = wpool.tile([128, 8], f32)
            nc.gpsimd.dma_start(out=bng_t, in_=d_bng[:, :])
            bnb_t = wpool.tile([128, 8], f32)
            nc.gpsimd.dma_start(out=bnb_t, in_=d_bnb[:, :])

        contrib = wpool.tile([128, PADQ, 256], fp8)

        nc.gpsimd.load_library(mlp_lib)
        # zero the unused contrib channels early (Pool is idle during conv1)
        nc.gpsimd.memset(contrib[:, :, 64:256], 0.0)
        # warm up the Relu activation table off the critical path
        warm = wpool.tile([128, 1], f32)
        nc.scalar.activation(warm, pb0_t[:, 0:1], AF.Relu)

        # ------------ conv1 (3x3 66->256 fp8 DR) + conv2 (1x1 256->64) ------------
        # linear-position tiles: out position L = 130*y + x (x<128 valid)
        with tc.tile_pool(name="stacks", bufs=1) as stpool, \
             tc.tile_pool(name="psumA", bufs=4, space="PSUM") as ppA, \
             tc.tile_pool(name="psumB", bufs=2, space="PSUM") as ppB, \
             tc.tile_pool(name="stage", bufs=3) as spool:
            st_t = stpool.tile([99, 2, PIMG], fp8)
            CK = PIMG // NCHUNK
            for c in range(NCHUNK):
                nc.sync.dma_start(out=st_t[:, :, c * CK:(c + 1) * CK],
                                  in_=d_st[:, :, c * CK:(c + 1) * CK])
            w1p_t = stpool.tile([99, 2, 3, 2, 128], fp8)
            nc.scalar.dma_start(out=w1p_t, in_=d_w1p[:, :, :, :, :])
            r1 = stpool.tile([128, 2, NL], fp8)

            def conv2_group(g):
                # 4 two-row px blocks; psum partition p = px pair rows
                # (260*b + 2p, +1) of feat_dram
                ps2 = ppB.tile([128, 512], f32, tag="psB", name="psB")
                for j in range(4):
                    b = 4 * g + j
                    for par in range(2):
                        lhsT = bass.AP(tensor=r1.tensor,
                                       offset=r1.offset + 260 * b + par,
                                       ap=[r1.ap[0], [NL, 2], [2, 128]])
                        nc.tensor.matmul(ps2[:, j * 128 + par * 64:
                                             j * 128 + par * 64 + 64],
                                         lhsT, w2t_t[:, :, :],
                                         start=True, stop=True, perf_mode=DR)
                stg = spool.tile([128, 512], f32, tag="stage", name="stg")
                if g % 2 == 0:
                    nc.scalar.activation(stg, ps2, AF.Copy)
                else:
                    nc.vector.tensor_copy(stg, ps2)
                dst = bass.AP(tensor=feat_dram, offset=g * 4 * 260 * 64,
                              ap=[[128, 128], [260 * 64, 4], [1, 128]])
                nc.sync.dma_start(out=dst, in_=stg)

            for t in range(33):          # linear tiles of 512 positions
                L0 = 512 * t
                for m in range(2):       # out-channel half
                    ps = ppA.tile([128, 512], f32, tag="psA", name="psA")
                    for kw in range(3):
                        rhs = bass.AP(tensor=st_t.tensor,
                                      offset=st_t.offset + L0 + kw,
                                      ap=[st_t.ap[0], [PIMG, 2], [1, 512]])
                        nc.tensor.matmul(ps, w1p_t[:, :, kw, m, :], rhs,
                                         start=(kw == 0), stop=(kw == 2),
                                         perf_mode=DR)
                    dst = r1[:, m, L0:L0 + 512]
                    ri = (t * 2 + m) % 11
                    if c1bias or ri < 5:
                        nc.scalar.activation(dst, ps, AF.Relu,
                                             bias=pb0_t[:, m:m + 1], scale=2.0)
                    elif ri < 9:
                        nc.vector.tensor_scalar(dst, ps, 2.0, 0.0,
                                                op0=ALU.mult, op1=ALU.max)
                    else:
                        nc.gpsimd.tensor_scalar(dst, ps, 2.0, 0.0,
                                                op0=ALU.mult, op1=ALU.max)
                # conv2 group g ready once relu covers 1040*g + 1041 positions
                if t >= 3 and t % 2 == 1:
                    conv2_group((t - 3) // 2)
            for g in range(15, 16):
                conv2_group(g)
            # catch-up: the 4 positions per 260-block not covered by M=[2,128]
            psx = ppB.tile([64, 256], f32, tag="psX", name="psX")
            for z, Lo in enumerate((256, 258)):
                for par in range(2):
                    lhsT = bass.AP(tensor=r1.tensor,
                                   offset=r1.offset + Lo + par,
                                   ap=[r1.ap[0], [NL, 2], [260, 64]])
                    nc.tensor.matmul(psx[:, (2 * z + par) * 64:
                                         (2 * z + par) * 64 + 64],
                                     lhsT, w2t_t[:, :, :],
                                     start=True, stop=True, perf_mode=DR)
            stg2 = spool.tile([64, 256], f32, tag="stage2", name="stg2")
            nc.scalar.activation(stg2, psx, AF.Copy)
            dstx = bass.AP(tensor=feat_dram, offset=256 * 64,
                           ap=[[260 * 64, 64], [128, 2], [1, 128]])
            nc.sync.dma_start(out=dstx, in_=stg2)

        # ------------ bilinear gather (2 row-pair gathers) + combine ------------
        with tc.tile_pool(name="gpool", bufs=1) as gpool:
            # snake weight loads (SP queue is free during the gather phase)
            nc.sync.dma_start(out=headw_t, in_=d_headw[:, :, :, :])
            nc.sync.dma_start(out=headb_t, in_=d_headb[:, :])
            nc.sync.dma_start(out=resw_t[:, 0:4], in_=d_resw[:, 0:4])
            nc.sync.dma_start(out=resw_t[:, 4:7], in_=d_resw[:, 4:7])
            nc.sync.dma_start(out=resb_t, in_=d_resb[:, :])
            nc.sync.dma_start(out=fusw_t, in_=d_fusw[:, :, :, :, :])
            nc.sync.dma_start(out=fusb_t, in_=d_fusb[:, :])
            nc.sync.dma_start(out=pw1_t, in_=d_pw1[:, :, :, :, :])
            nc.sync.dma_start(out=pb1_t, in_=d_pb1[:, :])
            nc.sync.dma_start(out=pw2_t, in_=d_pw2[:, :, :])
            nc.sync.dma_start(out=pb2_t, in_=d_pb2[:, :])
            nc.sync.dma_start(out=pw3_t, in_=d_pw3[:, :])
            gts = []
            src = bass.AP(tensor=feat_dram, offset=0, ap=[[64, NF - 1], [1, 128]])
            for yi, idxt in enumerate((idx0_t, idx1_t)):
                gt = gpool.tile([128, PADQ, 128], f32, tag=f"g{yi}", name=f"g{yi}")
                nc.gpsimd.dma_gather(gt, src, idxt[:, :], NVI, NVI, 128,
                                     elem_step=64, single_packet=False)
                gts.append(gt)
            va = gpool.tile([128, PADQ, 64], bf16, tag="va", name="va")
            vb = gpool.tile([128, PADQ, 64], bf16, tag="vb", name="vb")
            nc.vector.tensor_tensor(va, gts[0][:, :, 0:64],
                                    _bcast(wc_t[:, 0, 0, :], 64), ALU.mult)
            nc.vector.tensor_tensor(vb, gts[0][:, :, 64:128],
                                    _bcast(wc_t[:, 0, 1, :], 64), ALU.mult)
            nc.vector.tensor_tensor(va, va, vb, ALU.add)
            nc.vector.tensor_tensor(vb, gts[1][:, :, 0:64],
                                    _bcast(wc_t[:, 1, 0, :], 64), ALU.mult)
            nc.vector.tensor_tensor(va, va, vb, ALU.add)
            nc.vector.tensor_tensor(vb, gts[1][:, :, 64:128],
                                    _bcast(wc_t[:, 1, 1, :], 64), ALU.mult)
            if use_b2s:
                nc.vector.tensor_tensor(vb, vb, b2s_t, ALU.add)
            nc.vector.tensor_tensor(contrib[:, :, 0:64], va, vb, ALU.add)
            nc.vector.tensor_copy(contrib[:, :, 64:66], coords_t)

            # ---------------- snake (vertex-major: [ch, layer, k, q]) ----------
            with tc.tile_pool(name="snake", bufs=1) as sn, \
                 tc.tile_pool(name="psumS", bufs=4, space="PSUM") as ppS, \
                 tc.tile_pool(name="psumT", bufs=2, space="PSUM") as ppT:
                # transpose-gather contrib -> ipad (memory [128, NII, 2] fp8:
                # partition p plane j = channel 2p+j of position n = k*PADQ+q)
                ipad = sn.tile([128, 2, NII], fp8, tag="ipad", name="ipad")
                NI1 = 96 * PADQ
                for n0, n1 in ((0, NI1), (NI1, NII)):
                    odst = bass.AP(tensor=ipad.tensor,
                                   offset=ipad.offset + 2 * n0,
                                   ap=[ipad.ap[0], [n1 - n0, 2], [1, n1 - n0]])
                    nc.gpsimd.dma_gather(odst, contrib[:, :, :],
                                         iidx_t[:, 16 * n0 // 256:16 * n1 // 256],
                                         n1 - n0, n1 - n0, 256, transpose=True,
                                         single_packet=False,
                                         sbuf_tokens_per_rank=128,
                                         sbuf_free_dim_per_rank=256,
                                         sbuf_byte_offset=0)

                states = sn.tile([128, 8, PADV, PADQ], fp8, tag="states",
                                 name="states")
                ctrs = [sn.tile([128, 8, CB], fp8, tag=f"ctr{k}",
                                name=f"ctr{k}") for k in range(2)]

                def st_slice(li, k0, k1):
                    """states[:, li, k0:k1, :] as [128, (k1-k0)*PADQ]."""
                    return bass.AP(
                        tensor=states.tensor,
                        offset=states.offset + (li * PADV + k0) * PADQ,
                        ap=[states.ap[0], [1, (k1 - k0) * PADQ]])

                # head conv: K = 33 partitions x 2 (interleaved channel pairs)
                for vb in range(8):
                    ps = ppS.tile([128, CB], f32, tag="psS", name="psS")
                    for t in range(9):
                        off = 2 * ((16 + vb * 16 + (t - 4)) * PADQ)
                        rhs = bass.AP(tensor=ipad.tensor,
                                      offset=ipad.offset + off,
                                      ap=[[ipad.ap[0][0], 33], [1, 2],
                                          [2, CB]])
                        nc.tensor.matmul(ps, headw_t[:, t, :, :], rhs,
                                         start=(t == 0), stop=(t == 8),
                                         perf_mode=DR)
                    dst = st_slice(0, 16 + vb * 16, 32 + vb * 16)
                    if bn_extra:
                        cv = ctrs[0][:, vb, :]
                        nc.scalar.activation(cv, ps, AF.Relu,
                                             bias=headb_t[:, 0:1],
                                             scale=S_S / S_HEAD)
                        nc.vector.tensor_scalar(dst, cv,
                                                bng_t[:, 0:1], bnb_t[:, 0:1],
                                                op0=ALU.mult, op1=ALU.add)
                    else:
                        nc.scalar.activation(dst, ps, AF.Relu,
                                             bias=headb_t[:, 0:1],
                                             scale=S_S / S_HEAD)
                    if vb == 7:
                        nc.gpsimd.tensor_copy(st_slice(0, 0, 16),
                                              st_slice(0, 128, 144))
                    if vb == 0:
                        nc.gpsimd.tensor_copy(st_slice(0, 144, 160),
                                              st_slice(0, 16, 32))

                # res convs: 5 DR tap-pairs, fp8 states
                for i in range(NRES):
                    d = DIL[i]
                    ctr = ctrs[i % 2]
                    for jj in range(8):
                        vb = (i + 1 + jj) % 8   # rotate start for pipelining
                        ps = ppS.tile([128, CB], f32, tag="psS", name="psS")
                        for ti, t0 in enumerate((0, 2, 4, 6, 8)):
                            stride = d if t0 != 8 else 0
                            k0 = 16 + vb * 16 + (t0 - 4) * d
                            off = (i * PADV + k0) * PADQ
                            rhs = bass.AP(tensor=states.tensor,
                                          offset=states.offset + off,
                                          ap=[states.ap[0],
                                              [stride * PADQ, 2], [1, CB]])
                            nc.tensor.matmul(ps, resw_t[:, i, ti, :, :], rhs,
                                             start=(ti == 0), stop=(ti == 4),
                                             perf_mode=DR)
                        cv = ctr[:, vb, :]
                        if sb0 and jj >= 6:
                            nc.vector.tensor_scalar(cv, ps, S_S / S_RES,
                                                    0.0, op0=ALU.mult,
                                                    op1=ALU.max)
                        else:
                            nc.scalar.activation(cv, ps, AF.Relu,
                                                 bias=resb_t[:, i:i + 1],
                                                 scale=S_S / S_RES)
                        if bn_extra:
                            nc.vector.tensor_scalar(cv, cv, bng_t[:, i + 1:i + 2],
                                                    bnb_t[:, i + 1:i + 2],
                                                    op0=ALU.mult, op1=ALU.add)
                        radd = nc.gpsimd if jj % 2 == 0 else nc.vector
                        radd.tensor_tensor(
                            st_slice(i + 1, 16 + vb * 16, 32 + vb * 16), cv,
                            st_slice(i, 16 + vb * 16, 32 + vb * 16), ALU.add)
                        if vb == 7:
                            nc.gpsimd.tensor_copy(st_slice(i + 1, 0, 16),
                                                  st_slice(i + 1, 128, 144))
                        if vb == 0:
                            nc.gpsimd.tensor_copy(st_slice(i + 1, 144, 160),
                                                  st_slice(i + 1, 16, 32))

                # fusion 1x1 (1024->256) + per-poly max over V
                gmv = sn.tile([128, 2, 8, PADQ], f32, tag="gmv", name="gmv")
                gb = sn.tile([128, 2, PADQ], fp8, tag="gb", name="gb")
                gbx = sn.tile([128, 2, CB], fp8, tag="gbx", name="gbx")
                for m in range(2):
                    for vb in range(8):
                        ps = ppS.tile([128, CB], f32, tag="psS", name="psS")
                        for u in range(4):
                            off = (2 * u * PADV + 16 + vb * 16) * PADQ
                            rhs = bass.AP(tensor=states.tensor,
                                          offset=states.offset + off,
                                          ap=[states.ap[0], [SV, 2], [1, CB]])
                            nc.tensor.matmul(ps, fusw_t[:, u, :, m, :], rhs,
                                             start=(u == 0), stop=(u == 3),
                                             perf_mode=DR)
                        nc.vector.tensor_reduce(
                            gmv[:, m, vb, :],
                            bass.AP(tensor=ps.tensor, offset=ps.offset,
                                    ap=[ps.ap[0], [1, PADQ], [PADQ, 16]]),
                            axis=mybir.AxisListType.X, op=ALU.max)
                    # tree max over the 8 vertex blocks
                    nc.vector.tensor_tensor(gmv[:, m, 0:4, :], gmv[:, m, 0:4, :],
                                            gmv[:, m, 4:8, :], ALU.max)
                    nc.vector.tensor_tensor(gmv[:, m, 0:2, :], gmv[:, m, 0:2, :],
                                            gmv[:, m, 2:4, :], ALU.max)
                    nc.vector.tensor_tensor(gmv[:, m, 0:1, :], gmv[:, m, 0:1, :],
                                            gmv[:, m, 1:2, :], ALU.max)
                    nc.vector.tensor_scalar(gb[:, m, :], gmv[:, m, 0, :],
                                            S_G / S_FUS, fusb_t[:, m:m + 1],
                                            op0=ALU.mult, op1=ALU.add)
                    nc.vector.tensor_copy(
                        gbx[:, m, :],
                        bass.AP(tensor=gb.tensor, offset=gb.offset + m * PADQ,
                                ap=[gb.ap[0], [0, 16], [1, PADQ]]))

                # pred1: 1280 -> 256, relu -> h1 fp8 (vertex-major)
                h1 = sn.tile([128, 2, CV], fp8, tag="h1", name="h1")
                for m in range(2):
                    for vb in range(8):
                        ps = ppS.tile([128, CB], f32, tag="psS", name="psS")
                        rhs = bass.AP(tensor=gbx.tensor, offset=gbx.offset,
                                      ap=[gbx.ap[0], [CB, 2], [1, CB]])
                        nc.tensor.matmul(ps, pw1_t[:, 0, :, m, :], rhs,
                                         start=True, stop=False, perf_mode=DR)
                        for u in range(4):
                            off = (2 * u * PADV + 16 + vb * 16) * PADQ
                            rhs = bass.AP(tensor=states.tensor,
                                          offset=states.offset + off,
                                          ap=[states.ap[0], [SV, 2], [1, CB]])
                            nc.tensor.matmul(ps, pw1_t[:, u + 1, :, m, :], rhs,
                                             start=False, stop=(u == 3),
                                             perf_mode=DR)
                        if sb0 and vb % 8 >= 5:
                            nc.vector.tensor_scalar(
                                h1[:, m, vb * CB:(vb + 1) * CB], ps,
                                S_H1 / S_P1, 0.0,
                                op0=ALU.mult, op1=ALU.max)
                        else:
                            nc.scalar.activation(
                                h1[:, m, vb * CB:(vb + 1) * CB], ps, AF.Relu,
                                bias=pb1_t[:, m:m + 1], scale=S_H1 / S_P1)

                # pred2: 256 -> 64, relu -> h2 fp8
                h2 = sn.tile([64, CV], fp8, tag="h2", name="h2")
                for vb in range(8):
                    ps = ppT.tile([64, CB], f32, tag="psS2", name="psS2")
                    rhs = bass.AP(tensor=h1.tensor,
                                  offset=h1.offset + vb * CB,
                                  ap=[h1.ap[0], [CV, 2], [1, CB]])
                    nc.tensor.matmul(ps, pw2_t[:, :, :], rhs,
                                     start=True, stop=True, perf_mode=DR)
                    nc.scalar.activation(h2[:, vb * CB:(vb + 1) * CB], ps,
                                         AF.Relu, bias=pb2_t[:, 0:1],
                                         scale=S_H2 / S_P2)

                # pred3: 64 -> 2 per poly -> [128 v, PADQ, 2]
                ps3 = ppT.tile([128, PADQ * 2], f32, tag="psT3", name="psT3",
                               bufs=1)
                for q in range(PADQ):
                    lhsT = bass.AP(tensor=h2.tensor, offset=h2.offset + q,
                                   ap=[h2.ap[0], [PADQ, 128]])
                    nc.tensor.matmul(ps3[:, q * 2:(q + 1) * 2],
                                     lhsT, pw3_t[:, :], start=True, stop=True)
                o_t = sn.tile([128, PADQ, 2], f32, tag="o_t", name="o_t")
                nc.vector.scalar_tensor_tensor(
                    o_t, ps3.rearrange("p (a b) -> p a b", b=2), 1.0 / S_P3,
                    base_t, op0=ALU.mult, op1=ALU.add)
                nc.sync.dma_start(out=d_out[:, :, :], in_=o_t)

    nc.compile()
    return nc


_NC_CACHE = {}


def _get_nc(P, bn_extra=False, use_b2s=False, c1bias=False, sb0=True):
    key = (P, bn_extra, use_b2s, c1bias, sb0)
    if key not in _NC_CACHE:
        _NC_CACHE[key] = build_nc(P, bn_extra, use_b2s, c1bias, sb0)
    return _NC_CACHE[key]


def _flags(inputs):
    g_ok = (np.all(np.asarray(inputs["head_g"]) > 0)
            and np.all(np.asarray(inputs["res_g"]) > 0))
    b_zero = (np.all(np.asarray(inputs["head_bt"]) == 0)
              and np.all(np.asarray(inputs["res_bt"]) == 0))
    bn_extra = not (g_ok and b_zero)
    use_b2s = bool(np.any(np.asarray(inputs["proj_b2"]) != 0))
    c1bias = bool(np.any(np.asarray(inputs["proj_b1"]) != 0))
    sb0 = (np.all(np.asarray(inputs["head_b"]) == 0)
           and np.all(np.asarray(inputs["res_b"]) == 0)
           and np.all(np.asarray(inputs["pb1"]) == 0))
    return bn_extra, use_b2s, c1bias, sb0


def pack16(idx_flat, cols):
    tab = np.zeros((16, cols), np.int16)
    n = len(idx_flat)
    tab[np.arange(n) % 16, np.arange(n) // 16] = idx_flat.astype(np.int16)
    return np.tile(tab, (8, 1))


def _host_prep(inputs, P, counts, order, offs):
    """Build per-core in_maps."""
    bn_extra, use_b2s, c1bias, sb0 = _flags(inputs)
    cnn = np.asarray(inputs["cnn_feature"], np.float32)
    ipoly = np.asarray(inputs["i_it_poly"], np.float32)
    cpoly = np.asarray(inputs["c_it_poly"], np.float32)
    w1 = np.asarray(inputs["proj_w1"], np.float32)
    b1 = np.asarray(inputs["proj_b1"], np.float32)
    b2 = np.asarray(inputs["proj_b2"], np.float32)
    w2 = np.asarray(inputs["proj_w2"], np.float32)[:, :, 0, 0]  # [64, 256]
    PADQ = -(-P // 4) * 4
    NVI = PADQ * 128
    NII = PADQ * PADV

    # ---- grid-sample host math (fp32, matches reference) ----
    ix = ipoly[..., 0] - np.float32(0.5)
    iy = ipoly[..., 1] - np.float32(0.5)
    x0 = np.floor(ix); y0 = np.floor(iy)
    wx = (ix - x0).astype(np.float32); wy = (iy - y0).astype(np.float32)
    x0i = x0.astype(np.int64); y0i = y0.astype(np.int64)
    bx = np.clip(x0i, 0, W - 2)                      # gather row base (x)
    # feat rows are linear positions L = 130*y + x
    rows_y = []
    wslot = np.zeros((2, 2) + ipoly.shape[:2], np.float32)  # [yi, slot, NP, V]
    s_v = np.zeros(ipoly.shape[:2], np.float32)
    for yi in range(2):
        yc = y0i + yi
        yv = (yc >= 0) & (yc < H)
        ycc = np.clip(yc, 0, H - 1)
        rows_y.append(ycc * PADW + bx)
        wgt_y = np.where(yv, wy if yi else 1 - wy, 0.0)
        for s in range(2):
            xs = bx + s
            is_x0 = (xs == x0i) & (x0i >= 0)
            is_x1 = (xs == x0i + 1) & (x0i + 1 <= W - 1)
            wgt_x = np.where(is_x0, 1 - wx, np.where(is_x1, wx, 0.0))
            wslot[yi, s] = wgt_y * wgt_x
        s_v += wslot[yi, 0] + wslot[yi, 1]

    # ---- shared packed weights (fp8 with scale folding) ----
    def row_kh_ch(r):
        if r < 66:
            return 0, r
        if r < 132:
            return 1, r - 66
        return 2, r - 132
    w1p = np.zeros((99, 2, 3, 2, 128), np.float32)
    for r in range(198):
        p, j = r % 99, r // 99
        kh, ci = row_kh_ch(r)
        for kw in range(3):
            for m in range(2):
                w1p[p, j, kw, m, :] = w1[m * 128:(m + 1) * 128, ci, kh, kw] * S_W1
    w2t = (w2.T * (S_F / S_R1)).reshape(2, 128, 64).transpose(1, 0, 2)
    pb0 = (b1 * S_R1).reshape(2, 128).T.copy()

    head_g = np.asarray(inputs["head_g"], np.float32)
    res_g = np.asarray(inputs["res_g"], np.float32)
    gfold_h = head_g if not bn_extra else np.ones_like(head_g)
    gfold_r = res_g if not bn_extra else np.ones_like(res_g)

    # head: [33 part, 9 taps, 2 pair, 128 out]; channel of (p, j) = 2p+j
    hw_true = np.asarray(inputs["head_w"], np.float32)  # [128, 66, 9]
    sch = np.where(np.arange(66) < 64, S_V, S_C)
    headw = np.zeros((33, 9, 2, 128), np.float32)
    for p in range(33):
        for j in range(2):
            c = 2 * p + j
            headw[p, :, j, :] = (hw_true[:, c, :] * (S_HEAD / sch[c])
                                 * gfold_h[:, None]).T
    headb = (np.asarray(inputs["head_b"], np.float32) * S_S).reshape(128, 1)

    # res: tap pairs (0,1),(2,3),(4,5),(6,7),(8,dup-zero)
    rw_true = np.asarray(inputs["res_w"], np.float32)  # [7, 128, 128, 9]
    resw = np.zeros((128, 7, 5, 2, 128), np.float32)
    for i in range(7):
        wi = rw_true[i] * gfold_r[i][:, None, None]  # [o, c, tap]
        for ti, t0 in enumerate((0, 2, 4, 6, 8)):
            resw[:, i, ti, 0, :] = wi[:, :, t0].T * (S_RES / S_S)
            if t0 != 8:
                resw[:, i, ti, 1, :] = wi[:, :, t0 + 1].T * (S_RES / S_S)
    resb = (np.asarray(inputs["res_b"], np.float32) * S_S).T.copy()

    fw = np.asarray(inputs["fus_w"], np.float32).reshape(256, 8, 128)
    fusw = np.zeros((128, 4, 2, 2, 128), np.float32)
    for u in range(4):
        for j in range(2):
            for m in range(2):
                fusw[:, u, j, m, :] = (fw[m * 128:(m + 1) * 128, 2 * u + j, :]
                                       * (S_FUS / S_S)).T
    fusb = (np.asarray(inputs["fus_b"], np.float32) * S_G).reshape(2, 128).T.copy()

    p1 = np.asarray(inputs["pw1"], np.float32).reshape(256, 10, 128)
    pw1 = np.zeros((128, 5, 2, 2, 128), np.float32)
    for m in range(2):
        for j in range(2):
            pw1[:, 0, j, m, :] = (p1[m * 128:(m + 1) * 128, j, :]
                                  * (S_P1 / S_G)).T
        for u in range(4):
            for j in range(2):
                pw1[:, u + 1, j, m, :] = (p1[m * 128:(m + 1) * 128, 2 + 2 * u + j, :]
                                          * (S_P1 / S_S)).T
    pb1 = (np.asarray(inputs["pb1"], np.float32) * S_H1).reshape(2, 128).T.copy()
    p2 = np.asarray(inputs["pw2"], np.float32).reshape(64, 2, 128)
    pw2 = (p2 * (S_P2 / S_H1)).transpose(2, 1, 0).copy()
    pb2 = (np.asarray(inputs["pb2"], np.float32) * S_H2).reshape(64, 1)
    pw3 = (np.asarray(inputs["pw3"], np.float32).T * (S_P3 / S_H2)).copy()
    pb3 = np.asarray(inputs["pb3"], np.float32)

    shared = {
        "w1p": w1p.astype(E4), "w2t": w2t.astype(E4), "pb0": pb0,
        "headw": headw.astype(E4), "headb": headb,
        "resw": resw.astype(E4), "resb": resb,
        "fusw": fusw.astype(E4), "fusb": fusb,
        "pw1": pw1.astype(E4), "pb1": pb1,
        "pw2": pw2.astype(E4), "pb2": pb2, "pw3": pw3.astype(E4),
    }
    if bn_extra:
        bng = np.zeros((128, 8), np.float32)
        bnb = np.zeros((128, 8), np.float32)
        bng[:, 0] = head_g; bnb[:, 0] = np.asarray(inputs["head_bt"]) * S_S
        bng[:, 1:] = res_g.T; bnb[:, 1:] = np.asarray(inputs["res_bt"]).T * S_S
        shared["bng"] = bng; shared["bnb"] = bnb

    # iidx: ipad pos n = k*PADQ + q <- contrib token q, vertex (k+112)%128
    kk = np.arange(PADV)
    vv = (kk + 112) % 128
    iidx_full = (np.arange(PADQ)[None, :] * 128 + vv[:, None]).reshape(-1)

    ind = np.asarray(inputs["ind"]).astype(np.int64)
    in_maps = []
    for c in range(N_CORES):
        img = cnn[c]
        img_pad = np.zeros((C_IN, PADW, PADW), np.float32)
        img_pad[:, 1:129, 1:129] = img
        flat = img_pad.reshape(C_IN, PIMG)
        st = np.zeros((99, 2, PIMG), np.float32)
        for r in range(198):
            p, j = r % 99, r // 99
            kh, ci = row_kh_ch(r)
            if kh == 0:
                st[p, j] = flat[ci]
            else:
                st[p, j, :PIMG - 130 * kh] = flat[ci, 130 * kh:]

        own = order[offs[c]:offs[c + 1]]
        nown = len(own)
        idxs = np.zeros((2, NVI), np.int64)
        wc = np.zeros((128, 2, 2, PADQ), np.float32)
        for yi in range(2):
            idxs[yi, :nown * 128] = rows_y[yi][own].reshape(-1)
            for s in range(2):
                wc[:, yi, s, :nown] = (wslot[yi, s][own].T * (S_V / S_F))
        coords = np.zeros((128, PADQ, 2), np.float32)
        coords[:, :nown, :] = (cpoly[own] * (RO * S_C)).transpose(1, 0, 2)
        base = np.zeros((128, PADQ, 2), np.float32)
        if nown:
            base[:, :nown, :] = (ipoly[own] * RO + pb3[None, None, :]) \
                .transpose(1, 0, 2).astype(np.float32)

        m = {
            "st": st.astype(E4),
            "idx0": pack16(idxs[0], NVI // 16),
            "idx1": pack16(idxs[1], NVI // 16),
            "wc": wc, "coords": coords.astype(BF),
            "iidx": pack16(iidx_full, NII // 16),
            "base": base,
        }
        if use_b2s:
            b2s = np.zeros((128, PADQ, 64), np.float32)
            b2s[:, :nown, :] = s_v[own].T[:, :, None] * b2[None, None, :] * S_V
            m["b2s"] = b2s
        m.update(shared)
        in_maps.append(m)
    return in_maps


def kernel(**inputs):
    ind = np.asarray(inputs["ind"]).astype(np.int64)
    counts = np.bincount(ind, minlength=N_CORES)
    P = int(counts.max())
    assert P <= 32, f"per-image poly count {P} exceeds supported range"
    order = np.argsort(ind, kind="stable")
    offs = np.concatenate([[0], np.cumsum(counts)])

    bn_extra, use_b2s, c1bias, sb0 = _flags(inputs)
    nc = _get_nc(P, bn_extra, use_b2s, c1bias, sb0)
    in_maps = _host_prep(inputs, P, counts, order, offs)
    res = None
    last_err = None
    for _attempt in range(3):
        try:
            res = run_bass_kernel_spmd(nc, in_maps, list(range(N_CORES)))
            break
        except Exception as e:  # rare transient device error; retry
            last_err = e
    if res is None:
        raise last_err

    out = np.zeros((NP, V, 2), np.float32)
    for c in range(N_CORES):
        oc = res.results[c]["out"]  # [128v, PADQ, 2]
        own = order[offs[c]:offs[c + 1]]
        for q, opoly in enumerate(own):
            out[opoly] = oc[:, q, :]
    return out
